# revision 10
# baseline (speedup 1.0000x reference)
"""HMM log-likelihood (log-domain forward algorithm) on 8 Trainium2 cores.

Strategy: scaled linear-domain forward algorithm with warmup-halo sequence
parallelism, step-major host layout.  N=1e6 timesteps split into 8*CC
independent chains (CC/core); each chain starts from a uniform state W steps
before its owned region of L steps.  The host rearranges the per-core input
slice into X[s, partition, chain] (step-major), so the device DMA is large
contiguous packets that arrive in exactly the order the scan consumes them:
the scan overlaps the HBM stream instead of waiting behind it.

Per core, chains are batched 4-wide across the 128 SBUF partitions
(block-diagonal T^T weights on the PE) with the chain index in the matmul
free dimension; G=2 interleaved groups hide the matmul->multiply latency.
Each timestep is one bf16 matmul per group (T @ S into PSUM) plus one vector
multiply by the emission probabilities, split in half across the Vector and
GpSimd engines.  Exp (with folded bias) runs on the Scalar engine, windowed
behind the DMA.

Normalization is free: a constant per-step drift delta = E[log c] is folded
into the exp bias, making log|S| a zero-drift random walk, so no per-chain
rescaling is needed.  The bf16 quantization of T factors exactly as
D_r @ T_hat with T_hat row-stochastic; -log(r) is folded into the same exp
bias.  Each chain's contribution is log(sum(S_final)) - log(sum(S_at_W)) +
delta*L, assembled on the host, which also runs exact f64 scans for the
prefix [0, W) and the short tail.
"""

import sys

for p in ("/opt/trn_rl_repo", "/root/.axon_site", "/root/.axon_site/_ro/trn_rl_repo",
          "/root/.axon_site/_ro/pypackages"):
    if p not in sys.path:
        sys.path.insert(0, p)

import numpy as np

K = 32
N = 1_000_000
NCORES = 8
W = 4             # warmup (halo) steps per chain
L = 32            # owned steps per chain
CC = 3904         # chains per core
SPAN = W + L      # 36 sequential steps
CQ = CC // 4      # 976 chains per partition group
G = 2             # interleaved compute groups
F = CQ // G       # 488 chains (matmul free dim) per group
WINS = [1, 1] + [2] * 17   # per-window step counts (sum = SPAN)
assert sum(WINS) == SPAN
COVERED = W + NCORES * CC * L

_cache = {}


def _build():
    import concourse.bass as bass
    import concourse.bacc as bacc
    import concourse.mybir as mybir
    import concourse.tile as tile
    from contextlib import ExitStack

    f32 = mybir.dt.float32
    bf16 = mybir.dt.bfloat16
    fp8 = mybir.dt.float8e4
    AF = mybir.ActivationFunctionType

    nc = bacc.Bacc("TRN2", target_bir_lowering=False, debug=False,
                   num_devices=NCORES)
    # step-major input: X[s, p, j] with p = 32*q + k, chain = q*CQ + j
    x = nc.dram_tensor("x", [SPAN * 128, CQ], bf16, kind="ExternalInput")
    wmat = nc.dram_tensor("wmat", [128, 128], fp8, kind="ExternalInput")
    ebias = nc.dram_tensor("ebias", [128, 1], f32, kind="ExternalInput")
    snap_out = nc.dram_tensor("snap_out", [4, CQ], f32, kind="ExternalOutput")
    fin_out = nc.dram_tensor("fin_out", [4, CQ], f32, kind="ExternalOutput")

    with tile.TileContext(nc) as tc:
        with ExitStack() as ctx:
            cpool = ctx.enter_context(tc.tile_pool(name="const", bufs=1))
            rpool = ctx.enter_context(tc.tile_pool(name="rp", bufs=1))
            xpool = ctx.enter_context(tc.tile_pool(name="xp", bufs=4))
            spool = ctx.enter_context(tc.tile_pool(name="sp", bufs=2))
            pspool = ctx.enter_context(
                tc.tile_pool(name="ps", bufs=2, space=bass.MemorySpace.PSUM))

            w_t = cpool.tile([128, 128], fp8, tag="w")
            nc.sync.dma_start(w_t[:], wmat[:])
            ones_t = cpool.tile([128, 4], fp8, tag="ones")
            nc.vector.memset(ones_t[:], 0.0)
            for q in range(4):
                nc.vector.memset(ones_t[32 * q:32 * q + 32, q:q + 1], 1.0)
            eb_t = cpool.tile([128, 1], f32, tag="eb")
            nc.scalar.dma_start(eb_t[:], ebias[:])

            # all SPAN steps of emission data, step-major, exp'd in place
            rt = rpool.tile([128, SPAN, CQ], f32, tag="R")

            S, SN = [], []
            for g in range(G):
                st = spool.tile([128, F], bf16, tag=f"S{g}", name=f"st{g}")
                nc.vector.memset(st[:], 1.0)
                sn = cpool.tile([128, F], bf16, tag=f"N{g}")
                S.append(st)
                SN.append(sn)

            # window DMAs (4 per window, split by partition quarter) into a
            # bf16 staging tile, then exp -> the f32 R tile
            s0 = 0
            CH = CQ // 2
            for w, sb in enumerate(WINS):
                xt = xpool.tile([128, sb, CQ], bf16, tag=f"X{sb}",
                                name=f"xt{w}")
                if w < 4:
                    # early windows: 8 sub-DMAs (quarter x col-half) across
                    # all three issue queues for early engine parallelism
                    engs = [nc.sync, nc.gpsimd, nc.sync, nc.gpsimd,
                            nc.scalar, nc.sync, nc.gpsimd, nc.scalar]
                    for q in range(4):
                        for h in range(2):
                            src = bass.AP(
                                x, (s0 * 128 + 32 * q) * CQ + h * CH,
                                [[CQ, 32], [128 * CQ, sb], [1, CH]])
                            engs[q * 2 + h].dma_start(
                                xt[32 * q:32 * q + 32, :,
                                   h * CH:(h + 1) * CH], src)
                else:
                    for q in range(4):
                        src = bass.AP(x, (s0 * 128 + 32 * q) * CQ,
                                      [[CQ, 32], [128 * CQ, sb], [1, CQ]])
                        eng = nc.sync if (w * 4 + q) % 2 == 0 else nc.gpsimd
                        eng.dma_start(xt[32 * q:32 * q + 32, :, :], src)
                nc.scalar.activation(rt[:, s0:s0 + sb, :], xt[:],
                                     AF.Exp, bias=eb_t[:])
                s0 += sb

            for s in range(SPAN):
                for g in range(G):
                    ps = pspool.tile([128, F], f32, tag=f"mm{g}")
                    nc.tensor.matmul(ps[:], w_t[:], S[g][:], start=True,
                                     stop=True)
                    sn_new = spool.tile([128, F], bf16, tag=f"S{g}",
                                        name=f"st{g}_{s}")
                    c0 = g * F
                    nc.vector.tensor_mul(sn_new[:], ps[:],
                                         rt[:, s, c0:c0 + F])
                    S[g] = sn_new
                    if s == W - 1:
                        nc.gpsimd.tensor_copy(SN[g][:], S[g][:])
                if s == W:
                    # off-chain: 32->1 partition sums of the snapshot
                    for g in range(G):
                        pss = pspool.tile([4, F], f32, tag=f"sn{g}")
                        nc.tensor.matmul(pss[:], ones_t[:], SN[g][:],
                                         start=True, stop=True)
                        sns = cpool.tile([4, F], f32, tag=f"sns{g}")
                        nc.scalar.copy(sns[:], pss[:])
                        nc.sync.dma_start(
                            snap_out[:, g * F:(g + 1) * F], sns[:])

            for g in range(G):
                psf = pspool.tile([4, F], f32, tag=f"sn{g}")
                nc.tensor.matmul(psf[:], ones_t[:], S[g][:],
                                 start=True, stop=True)
                fns = cpool.tile([4, F], f32, tag=f"fns{g}")
                nc.scalar.copy(fns[:], psf[:])
                eng = nc.scalar if g == 0 else nc.gpsimd
                eng.dma_start(fin_out[:, g * F:(g + 1) * F], fns[:])

    nc.compile()
    return nc


def _get_nc():
    if "nc" not in _cache:
        _cache["nc"] = _build()
    return _cache["nc"]


def _log_softmax64(v, axis):
    v = v.astype(np.float64)
    m = v.max(axis=axis, keepdims=True)
    e = np.exp(v - m)
    return v - m - np.log(e.sum(axis=axis, keepdims=True))


def _estimate_delta(log_pdf, T64):
    # E[log c] from a vectorized short scan: 64 parallel probes, 56 steps,
    # burn-in 16 (mixing time is ~10 steps).
    NCH, NST, BURN = 64, 56, 16
    cols = np.arange(NCH) * 997 + 1
    a = np.full((K, NCH), 1.0 / K)
    samples = []
    for s in range(NST):
        p = np.exp(log_pdf[:, cols + s].astype(np.float64))
        a = p * (T64 @ a)
        c = a.sum(axis=0)
        a /= c
        if s >= BURN:
            samples.append(np.log(c))
    return float(np.mean(samples))


def _make_in_maps(log_pdf, T64):
    from ml_dtypes import bfloat16, float8_e4m3fn

    T32 = T64.astype(np.float32)
    Tq = T32.astype(float8_e4m3fn)
    delta = _estimate_delta(log_pdf, T64)
    # fp8-quantized T is exactly D_r @ T_hat with T_hat row-stochastic and
    # r the fp8 row sums; fold -log(r) and the drift -delta into the exp.
    r = Tq.astype(np.float64).sum(axis=1)
    eb = np.zeros((128, 1), dtype=np.float32)
    for q in range(4):
        eb[32 * q:32 * q + 32, 0] = (-np.log(r) - delta).astype(np.float32)
    wm = np.zeros((128, 128), dtype=float8_e4m3fn)
    for q in range(4):
        wm[32 * q:32 * q + 32, 32 * q:32 * q + 32] = Tq.T

    # step-major gather: X[s, 32q+k, j] = log_pdf[k, m*CC*L + (q*CQ+j)*L + s]
    col = np.arange(CC, dtype=np.int64) * L          # [CC]
    step = np.arange(SPAN, dtype=np.int64)           # [SPAN]
    idx0 = col[None, :] + step[:, None]              # [SPAN, CC]
    in_maps = []
    for m in range(NCORES):
        idx = m * CC * L + idx0
        xm = log_pdf[:, idx]                         # [K, SPAN, CC]
        # -> [SPAN, 4, K, CQ] -> [SPAN*128, CQ]
        xm = xm.reshape(K, SPAN, 4, CQ).transpose(1, 2, 0, 3)
        xm = np.ascontiguousarray(xm.astype(bfloat16)).reshape(SPAN * 128, CQ)
        in_maps.append({"x": xm, "wmat": wm, "ebias": eb})

    return in_maps, delta


def kernel(log_pdf: np.ndarray, pi: np.ndarray, T: np.ndarray) -> np.ndarray:
    from concourse.bass_utils import run_bass_kernel_spmd

    log_pdf = np.ascontiguousarray(log_pdf, dtype=np.float32)
    log_pi64 = _log_softmax64(pi, 0)
    log_T64 = _log_softmax64(T, 1)
    T64 = np.exp(log_T64)                     # row-stochastic [K, K] f64

    in_maps, delta = _make_in_maps(log_pdf, T64)
    nc = _get_nc()
    res = run_bass_kernel_spmd(nc, in_maps, list(range(NCORES))).results

    # ---- host combine (f64) ----
    LP = log_pdf
    # exact prefix [0, W)
    a = np.exp(log_pi64 + LP[:, 0].astype(np.float64))
    c = a.sum()
    total = np.log(c)
    a /= c
    for t in range(1, W):
        a = np.exp(LP[:, t].astype(np.float64)) * (T64 @ a)
        c = a.sum()
        total += np.log(c)
        a /= c

    # per-chain contributions: log(sum fin) - log(sum snap) + delta*L
    for m in range(NCORES):
        ssum = res[m]["snap_out"].astype(np.float64)   # [4, CQ]
        fsum = res[m]["fin_out"].astype(np.float64)
        total += (np.log(fsum) - np.log(ssum)).sum() + delta * L * CQ * 4

    # exact tail [COVERED, N) from the last covered column's true filter:
    # recompute it exactly on the host over the last chain's span
    mlast = NCORES - 1
    c_last = mlast * CC * L + (CC - 1) * L
    a = np.full(K, 1.0 / K)
    for t in range(c_last, COVERED):
        a = np.exp(LP[:, t].astype(np.float64)) * (T64 @ a)
        a /= a.sum()
    for t in range(COVERED, N):
        a = np.exp(LP[:, t].astype(np.float64)) * (T64 @ a)
        c = a.sum()
        total += np.log(c)
        a /= c

    return np.float32(total)
